# revision 11
# baseline (speedup 1.0000x reference)
"""Trainium2 Bass kernel for CohereAttention (QK-LayerNorm + interleaved RoPE +
GQA sliding-window attention), sharded over 8 NeuronCores.

Sharding: tensor-parallel over Q heads (4 per core); with H//KVH == 4 each core
owns exactly one KV head. Attention outputs are exchanged with an AllToAll
(token-major blocks) and o_proj is token-parallel: each core computes the full
4096-wide o_proj output for its 256-token slice per batch, streaming wo ONCE
(shared across both batches).

Key engine/scheduling decisions (from NTFF trace analysis):
  - All matmuls contract over the partition axis; weights are host-retiled to
    partition-major contiguous blocks so every DMA moves large contiguous
    lines (the [c p]-interleaved layouts cost ~2x DMA efficiency and ~0.8us
    of issue time per dma_start).
  - DVE RECIPROCAL is ~16 cyc/elem on TRN2 -- all 1/x go through ACT Ln+Exp
    (or are split ACT/DVE in attention to balance engine load).
  - Partition broadcasts are free via matmul: stationary operands (winv, ones)
    are replicated to 128 identical columns, so the [128, T] result of the
    sum-of-squares / softmax-denominator matmul is already broadcast.
    gpsimd is left entirely to the collectives.
  - Attention processes two heads in a software pipeline (PV/ones lag the
    scores by one pair) so the PE never sits behind the ACT exp chain.
  - o_proj pairs output blocks (ob, ob+4) so each af stationary load feeds
    two 512-wide matmuls (12.5% LDWEIGHTS tax instead of 25%), streams wo
    once for both batches, and runs batch-0 chains first so the batch-1
    AllToAll completes in the shadow.
  - DMA queues: hT on sync, weights/wo on scalar, a2a writes/af/out on vector.
"""

import sys

sys.path.insert(0, "/opt/trn_rl_repo")

import numpy as np
import ml_dtypes

import concourse.bass as bass
import concourse.mybir as mybir
import concourse.tile as tile
from concourse import bacc
from concourse.bass import ts, ds
from concourse.bass_utils import run_bass_kernel_spmd

B, S, H, KVH, D, HID = 2, 2048, 32, 8, 128, 4096
WINDOW = 512
EPS = 1e-5
SCALE = float(D) ** -0.5
NC = 8
HPC = H // NC              # q heads per core (4)
QW = HPC * D               # q width per core (512)
FCH = HID // 128           # contraction chunks (32)
TT = 512                   # projection token tile
NTT = S // TT              # 4
QT = 256                   # attention query tile
NKC = (WINDOW + QT) // 128  # key chunks per query tile window (6)
TSL = S // NC              # tokens per (core, batch) slice for o_proj (256)
NOB = HID // 512           # o_proj output blocks (8)

BF16 = mybir.dt.bfloat16
F32 = mybir.dt.float32
npbf16 = ml_dtypes.bfloat16

SWAP32 = [i ^ 1 for i in range(32)]  # adjacent-pair partition swap

_CACHE = {}


def _edge_masks():
    jj = np.arange(128)[:, None]
    qi = np.arange(QT)[None, :]

    def m(off):
        u = off + qi - jj
        return ((u >= 0) & (u < WINDOW)).astype(npbf16)

    mw = np.concatenate([m(512), m(384)], axis=1)   # chunks kk=0,1 (window edge)
    mc = np.concatenate([m(0), m(-128)], axis=1)    # chunks kk=4,5 (causal edge)
    return mw, mc


def _build_module():
    nc = bacc.Bacc(
        "TRN2",
        target_bir_lowering=False,
        debug=False,
        enable_asserts=False,
        num_devices=NC,
    )

    # host-retiled inputs: everything partition-major / contiguous per DMA tile
    hT = nc.dram_tensor("hT", [B, FCH, NTT, 128, TT], BF16,
                        kind="ExternalInput").ap()
    cosT = nc.dram_tensor("cosT", [B, D, S], BF16, kind="ExternalInput").ap()
    sinT = nc.dram_tensor("sinT", [B, D, S], BF16, kind="ExternalInput").ap()
    wq = nc.dram_tensor("wq", [128, FCH, QW], BF16, kind="ExternalInput").ap()
    wk = nc.dram_tensor("wk", [128, FCH, D], BF16, kind="ExternalInput").ap()
    wv = nc.dram_tensor("wv", [128, FCH, D], BF16, kind="ExternalInput").ap()
    wo = nc.dram_tensor("wo", [NOB, 128, FCH, 512], BF16,
                        kind="ExternalInput").ap()
    winvq = nc.dram_tensor("winvq", [D, 128], BF16, kind="ExternalInput").ap()
    winvk = nc.dram_tensor("winvk", [D, 128], BF16, kind="ExternalInput").ap()
    out = nc.dram_tensor("out", [B, TSL // 128, NOB, 128, 512], F32,
                         kind="ExternalOutput").ap()

    a2ain = {
        (b, g): nc.dram_tensor(f"a2ain{b}{g}", [NC, 2, 128, TSL], BF16,
                               kind="Internal").ap()
        for b in range(B) for g in range(HPC // 2)
    }
    a2aout = {
        (b, g): nc.dram_tensor(f"a2aout{b}{g}", [NC, 2, 128, TSL], BF16,
                               kind="Internal").ap()
        for b in range(B) for g in range(HPC // 2)
    }

    ones_d = nc.inline_tensor(np.ones((128, 128), dtype=npbf16),
                              name="ones128").ap()
    mw_np, mc_np = _edge_masks()
    maskw_d = nc.inline_tensor(mw_np, name="maskw").ap()
    maskc_d = nc.inline_tensor(mc_np, name="maskc").ap()

    rg = [list(range(NC))]

    with tile.TileContext(nc) as tc, \
            tc.tile_pool(name="sb", bufs=1) as sb, \
            tc.tile_pool(name="ps", bufs=1, space="PSUM") as ps:

        # --- resident weights / constants ---
        # First f-blocks of wq/wk/wv ride the Sync queue so the first
        # projection matmuls start within a few us; everything else goes on
        # the Scalar queue so it never blocks the streamed hT tiles.
        wq_sb = sb.tile([128, FCH, QW], BF16, tag="wq", bufs=1, name="wq_sb")
        wk_sb = sb.tile([128, FCH, D], BF16, tag="wk", bufs=1, name="wk_sb")
        wv_sb = sb.tile([128, FCH, D], BF16, tag="wv", bufs=1, name="wv_sb")
        nc.sync.dma_start(wq_sb[:, ds(0, 4), :], wq[:, ds(0, 4), :])
        nc.sync.dma_start(wk_sb[:, ds(0, 4), :], wk[:, ds(0, 4), :])
        nc.sync.dma_start(wv_sb[:, ds(0, 4), :], wv[:, ds(0, 4), :])
        # f4..31 stream inline with the first token tile (need-order) so the
        # PE never waits ~20us for the full weight preload
        ones_sb = sb.tile([128, 128], BF16, tag="ones", bufs=1, name="ones_sb")
        nc.scalar.dma_start(ones_sb[:], ones_d)
        maskw_sb = sb.tile([128, 2 * QT], BF16, tag="maskw", bufs=1,
                           name="maskw_sb")
        nc.scalar.dma_start(maskw_sb[:], maskw_d)
        maskc_sb = sb.tile([128, 2 * QT], BF16, tag="maskc", bufs=1,
                           name="maskc_sb")
        nc.scalar.dma_start(maskc_sb[:], maskc_d)
        winvq_sb = sb.tile([D, 128], BF16, tag="winvq", bufs=1, name="winvq_sb")
        nc.scalar.dma_start(winvq_sb[:], winvq)
        winvk_sb = sb.tile([D, 128], BF16, tag="winvk", bufs=1, name="winvk_sb")
        nc.scalar.dma_start(winvk_sb[:], winvk)
        eps_sb = sb.tile([128, 1], F32, tag="eps", bufs=1, name="eps_sb")
        nc.vector.memset(eps_sb[:], EPS)

        def ln_rope(qsb, winv_sb, cos_sb, sin_sb, tt, dst, sn):
            """LayerNorm (mean pre-folded on host) + interleaved RoPE on a
            drained [d, TT] bf16 tile; writes bf16 into dst[:, tt*TT:...].

            rstd = exp(-0.5*ln(ssq/D + eps)) on ACT -- DVE reciprocal is
            ~16 cyc/elem, Ln+Exp are ~1 cyc/col.  ssq comes out of the PE
            already broadcast to 128 partitions because winv_sb has 128
            identical columns (matmul cost only depends on the free size)."""
            sq = sb.tile([128, TT], BF16, tag="sq", bufs=2, name="sq")
            nc.vector.tensor_mul(sq[:], qsb[:], qsb[:])
            ssq = ps.tile([128, TT], F32, tag=f"s{sn}", bufs=1, name="ssq")
            nc.tensor.matmul(ssq[:], winv_sb[:], sq[:], start=True, stop=True)
            lt = sb.tile([128, TT], F32, tag="lt", bufs=2, name="lt")
            nc.scalar.activation(
                lt[:], ssq[:], mybir.ActivationFunctionType.Ln,
                bias=eps_sb[:], scale=1.0 / D,
            )
            rstd = sb.tile([128, TT], BF16, tag="rstd", bufs=2, name="rstd")
            nc.scalar.activation(
                rstd[:], lt[:], mybir.ActivationFunctionType.Exp, scale=-0.5,
            )
            qn = sb.tile([128, TT], BF16, tag="qn", bufs=2, name="qn")
            nc.vector.tensor_mul(qn[:], qsb[:], rstd[:])
            qs = sb.tile([128, TT], BF16, tag="qs", bufs=2, name="qs")
            nc.vector.stream_shuffle(qs[:], qn[:], SWAP32)
            t1 = sb.tile([128, TT], BF16, tag="t1", bufs=2, name="t1")
            nc.vector.tensor_mul(t1[:], qn[:], cos_sb[:, ts(tt, TT)])
            t2 = sb.tile([128, TT], BF16, tag="t2", bufs=2, name="t2")
            nc.vector.tensor_mul(t2[:], qs[:], sin_sb[:, ts(tt, TT)])
            nc.vector.tensor_add(dst[:, ts(tt, TT)], t1[:], t2[:])

        qT = {}   # (b, h) -> [128, S] bf16 rope'd normalized q, transposed
        kT = {}   # b -> [128, S]
        Vn = {}   # b -> [128, S//128, 128] natural [j, d] chunks
        vT = {}   # b -> [128, S] transposed v (pre transpose)
        trig = {}  # b -> (cos_sb, sin_sb)

        def proj_setup(b):
            cos_sb = sb.tile([128, S], BF16, tag="cos", bufs=1, name="cos_sb")
            nc.scalar.dma_start(cos_sb[:], cosT[b])
            sin_sb = sb.tile([128, S], BF16, tag="sin", bufs=1, name="sin_sb")
            nc.scalar.dma_start(sin_sb[:], sinT[b])
            trig[b] = (cos_sb, sin_sb)
            for h in range(HPC):
                qT[(b, h)] = sb.tile([128, S], BF16, tag="qT", bufs=4,
                                     name=f"qT{b}{h}")
            kT[b] = sb.tile([128, S], BF16, tag="kT", bufs=2, name=f"kT{b}")
            vT[b] = sb.tile([128, S], BF16, tag="vT", bufs=2, name=f"vT{b}")
            Vn[b] = sb.tile([128, S // 128, 128], BF16, tag="Vn", bufs=2,
                            name=f"Vn{b}")

        def proj_tt(b, tt):
            with nc.named_scope(f"proj_b{b}"):
                cos_sb, sin_sb = trig[b]
                qps = [
                    ps.tile([128, TT], F32, tag=f"acc{i}", bufs=1,
                            name=f"qps{i}")
                    for i in range(HPC)
                ]
                kps = ps.tile([128, TT], F32, tag="acck", bufs=1, name="kps")
                vps = ps.tile([128, TT], F32, tag="accv", bufs=1, name="vps")
                for f in range(FCH):
                    if b == 0 and tt == 0 and f >= 4:
                        # need-order weight streaming, alternating queues
                        eng = nc.sync if f % 2 == 0 else nc.scalar
                        eng.dma_start(wq_sb[:, f, :], wq[:, f, :])
                        eng.dma_start(wk_sb[:, f, :], wk[:, f, :])
                        eng.dma_start(wv_sb[:, f, :], wv[:, f, :])
                    ht_t = sb.tile([128, TT], BF16, tag="ht", bufs=7,
                                   name="ht_t")
                    if f % 2 == 0:
                        nc.sync.dma_start(ht_t[:], hT[b, f, tt])
                    else:
                        nc.scalar.dma_start(ht_t[:], hT[b, f, tt])
                    st = f == 0
                    sp = f == FCH - 1
                    for h in range(HPC):
                        nc.tensor.matmul(
                            qps[h][:], wq_sb[:, f, ts(h, D)], ht_t[:],
                            start=st, stop=sp,
                        )
                    nc.tensor.matmul(kps[:], wk_sb[:, f, :], ht_t[:],
                                     start=st, stop=sp)
                    nc.tensor.matmul(vps[:], wv_sb[:, f, :], ht_t[:],
                                     start=st, stop=sp)
                # Drain all six PSUM banks, alternating engines, so the next
                # tile's accumulation chains unblock as early as possible.
                qsb = []
                for i in range(HPC):
                    q = sb.tile([128, TT], BF16, tag="qsb", bufs=4,
                                name=f"qsb{i}")
                    if i % 2 == 0:
                        nc.scalar.copy(q[:], qps[i][:])
                    else:
                        nc.vector.tensor_copy(q[:], qps[i][:])
                    qsb.append(q)
                ksb = sb.tile([128, TT], BF16, tag="qsb", bufs=4, name="ksb")
                nc.scalar.copy(ksb[:], kps[:])
                nc.vector.tensor_copy(vT[b][:, ts(tt, TT)], vps[:])
                for h in range(HPC):
                    ln_rope(qsb[h], winvq_sb, cos_sb, sin_sb, tt, qT[(b, h)],
                            h % 2)
                ln_rope(ksb, winvk_sb, cos_sb, sin_sb, tt, kT[b], 0)

        def proj_vtrans(b):
            # transpose v to natural [j, d] chunk layout via the DMA xbar
            with nc.named_scope(f"proj_b{b}"):
                nc.scalar.dma_start_transpose(Vn[b][:], vT[b][:])

        def attn_heads(b, h0):
            """Two heads (h0, h0+1) interleaved; PV/ones lag scores by one
            pair so the PE stream never waits on the ACT exp."""
            with nc.named_scope(f"attn_b{b}"):
                streams = []
                for si, h in enumerate((h0, h0 + 1)):
                    attn_sb = sb.tile([128, S], BF16, tag=f"attn{si}", bufs=2,
                                      name=f"attn_sb{si}")
                    streams.append({"h": h, "si": si, "attn": attn_sb})

                def scores(st, qt, pp, kstart):
                    si, h = st["si"], st["h"]
                    i0 = qt * QT
                    kk = kstart + 2 * pp
                    j0 = i0 - WINDOW + kk * 128
                    sps = ps.tile([128, 2 * QT], F32,
                                  tag=f"acc{2 * si + pp % 2}", bufs=1,
                                  name="sps")
                    nc.tensor.matmul(
                        sps[:, 0:QT], kT[b][:, ds(j0, 128)],
                        qT[(b, h)][:, ds(i0, QT)],
                        start=True, stop=True,
                    )
                    nc.tensor.matmul(
                        sps[:, QT:2 * QT], kT[b][:, ds(j0 + 128, 128)],
                        qT[(b, h)][:, ds(i0, QT)],
                        start=True, stop=True,
                    )
                    pt = sb.tile([128, 2 * QT], BF16, tag=f"pt{si}", bufs=2,
                                 name="pt")
                    nc.scalar.activation(
                        pt[:], sps[:], mybir.ActivationFunctionType.Exp,
                        scale=SCALE,
                    )
                    if kk == 0:      # window edge pair (kk=0,1)
                        nc.vector.tensor_mul(pt[:], pt[:], maskw_sb[:])
                    elif kk == 4:    # causal edge pair (kk=4,5)
                        nc.vector.tensor_mul(pt[:], pt[:], maskc_sb[:])
                    return pt

                def pv(st, qt, pp, kstart, npair, pt, ops, lps):
                    i0 = qt * QT
                    kk = kstart + 2 * pp
                    j0 = i0 - WINDOW + kk * 128
                    first = pp == 0
                    last = pp == npair - 1
                    jc = j0 // 128
                    nc.tensor.matmul(
                        ops[:], Vn[b][:, jc, :], pt[:, 0:QT],
                        start=first, stop=False,
                    )
                    nc.tensor.matmul(
                        ops[:], Vn[b][:, jc + 1, :], pt[:, QT:2 * QT],
                        start=False, stop=last,
                    )
                    # softmax denominator, broadcast to all 128 partitions by
                    # the 128-identical-column ones stationary
                    nc.tensor.matmul(lps[:], ones_sb[:], pt[:, 0:QT],
                                     start=first, stop=False)
                    nc.tensor.matmul(lps[:], ones_sb[:], pt[:, QT:2 * QT],
                                     start=False, stop=last)

                def finish_qt(st, qt, ops, lps):
                    # Drain ops to SBUF immediately (frees the PSUM bank so
                    # the next tile's PV chain starts without waiting for the
                    # denominator), then normalize from SBUF.
                    si = st["si"]
                    i0 = qt * QT
                    osb = sb.tile([128, QT], F32, tag=f"osb{si}", bufs=2,
                                  name="osb")
                    nc.vector.tensor_copy(osb[:], ops[:])
                    linv = sb.tile([128, QT], F32, tag=f"linv{si}", bufs=2,
                                   name="linv")
                    if si == 0:
                        # ACT: linv = exp(-ln(l))
                        lt = sb.tile([128, QT], F32, tag="alt", bufs=2,
                                     name="alt")
                        nc.scalar.activation(
                            lt[:], lps[:], mybir.ActivationFunctionType.Ln,
                        )
                        nc.scalar.activation(
                            linv[:], lt[:], mybir.ActivationFunctionType.Exp,
                            scale=-1.0,
                        )
                    else:
                        # DVE reciprocal for the other stream (engine balance)
                        nc.vector.reciprocal(linv[:], lps[:])
                    nc.vector.tensor_mul(st["attn"][:, ds(i0, QT)], osb[:],
                                         linv[:])

                for qt in range(S // QT):
                    i0 = qt * QT
                    kstart = max(0, (WINDOW - i0) // 128)
                    npair = (NKC - kstart) // 2
                    ops = {}
                    lps = {}
                    for st in streams:
                        si = st["si"]
                        ops[si] = ps.tile([128, QT], F32,
                                          tag="acck" if si == 0 else "accv",
                                          bufs=1, name="ops")
                        lps[si] = ps.tile([128, QT], F32,
                                          tag="s0" if si == 0 else "s1",
                                          bufs=1, name="lps")
                    pts = {0: {}, 1: {}}
                    for pp in range(npair):
                        for st in streams:
                            pts[st["si"]][pp] = scores(st, qt, pp, kstart)
                        if pp > 0:
                            for st in streams:
                                si = st["si"]
                                pv(st, qt, pp - 1, kstart, npair,
                                   pts[si][pp - 1], ops[si], lps[si])
                                del pts[si][pp - 1]
                    for st in streams:
                        si = st["si"]
                        pv(st, qt, npair - 1, kstart, npair,
                           pts[si][npair - 1], ops[si], lps[si])
                    for st in streams:
                        finish_qt(st, qt, ops[st["si"]], lps[st["si"]])
                # ship both heads' outputs: one contiguous 64KB DMA per
                # (head, peer) on the gpsimd queue -- the collective that
                # consumes them lives there too, so ordering is natural and
                # the sync/scalar queues stay free for the next phase's
                # prefetches.
                for st in streams:
                    for n in range(NC):
                        nc.gpsimd.dma_start(
                            a2ain[(b, h0 // 2)][n, st["h"] - h0],
                            st["attn"][:, ts(n, TSL)],
                        )

        def a2a_phase(b, g):
            nc.gpsimd.collective_compute(
                "AllToAll",
                mybir.AluOpType.bypass,
                replica_groups=rg,
                ins=[a2ain[(b, g)][:]],
                outs=[a2aout[(b, g)][:]],
            )

        af = {}

        def af_load(b, eng):
            af[b] = sb.tile([128, 2, NC, 2, TSL], BF16, tag=f"af{b}", bufs=1,
                            name=f"af{b}")
            for g in range(HPC // 2):
                eng.dma_start(
                    af[b][:, g],
                    a2aout[(b, g)].rearrange("n c p t -> p n c t"),
                )

        PTAGS = ["acc0", "acc1", "acc2", "acc3", "acck", "accv", "s0", "s1"]

        def oproj_obp(obp, groups):
            # one output-block pair (ob, ob+4) of o_proj for the given
            # (batch, t2) groups.  wo streams in contiguous 2-chunk pieces
            # alternating scalar/sync; each af stationary chunk feeds two
            # 512-wide matmuls (halved LDWEIGHTS tax).  c-outer piece order
            # means pieces are consumed sequentially -> small bufs stream.
            oblo, obhi = obp, obp + NOB // 2
            wo_lo = []
            wo_hi = []
            for i in range(FCH // 2):
                wl = sb.tile([128, 2, 512], BF16, tag="wolo", bufs=3,
                             name="wo_lo")
                wh = sb.tile([128, 2, 512], BF16, tag="wohi", bufs=3,
                             name="wo_hi")
                if i % 2 == 0:
                    nc.scalar.dma_start(wl[:], wo[oblo][:, ds(2 * i, 2), :])
                    nc.sync.dma_start(wh[:], wo[obhi][:, ds(2 * i, 2), :])
                else:
                    nc.sync.dma_start(wl[:], wo[oblo][:, ds(2 * i, 2), :])
                    nc.scalar.dma_start(wh[:], wo[obhi][:, ds(2 * i, 2), :])
                wo_lo.append(wl)
                wo_hi.append(wh)
            po = {}
            for gi, (bb, t2) in enumerate(groups):
                po[(bb, t2)] = (
                    ps.tile([128, 512], F32, tag=PTAGS[2 * gi], bufs=1,
                            name="plo"),
                    ps.tile([128, 512], F32, tag=PTAGS[2 * gi + 1], bufs=1,
                            name="phi"),
                )
            for pc in range(FCH // 2):
                for cc in range(2):
                    c = 2 * pc + cc
                    st = c == 0
                    sp = c == FCH - 1
                    for bb, t2 in groups:
                        ch = c % HPC
                        stat = af[bb][:, ch // 2, c // HPC, ch % 2,
                                      ds(t2 * 128, 128)]
                        p_lo, p_hi = po[(bb, t2)]
                        nc.tensor.matmul(p_lo[:], stat, wo_lo[pc][:, cc, :],
                                         start=st, stop=sp)
                        nc.tensor.matmul(p_hi[:], stat, wo_hi[pc][:, cc, :],
                                         start=st, stop=sp)
            for gi, (bb, t2) in enumerate(groups):
                p_lo, p_hi = po[(bb, t2)]
                for ob, pp in ((oblo, p_lo), (obhi, p_hi)):
                    ot = sb.tile([128, 512], F32, tag="ot", bufs=2, name="ot")
                    if (t2 + ob) % 2 == 0:
                        nc.scalar.copy(ot[:], pp[:])
                    else:
                        nc.vector.tensor_copy(ot[:], pp[:])
                    if ob % 2 == 0:
                        nc.sync.dma_start(out[bb, t2, ob], ot[:])
                    else:
                        nc.scalar.dma_start(out[bb, t2, ob], ot[:])

        def oproj_phase():
            with nc.named_scope("oproj"):
                # batch-0-only pre-pass on obp0 hides the batch-1 AllToAll;
                # then the fused remainder streams wo once for both batches
                oproj_obp(0, [(0, 0), (0, 1)])
                oproj_obp(0, [(1, 0), (1, 1)])
                for obp in range(1, NOB // 2):
                    oproj_obp(obp, [(0, 0), (0, 1), (1, 0), (1, 1)])

        proj_setup(0)
        for tt in range(NTT):
            proj_tt(0, tt)
        proj_vtrans(0)
        proj_setup(1)    # cos/sin for batch 1 load during attn0
        for h0 in range(0, HPC, 2):
            attn_heads(0, h0)
            a2a_phase(0, h0 // 2)   # first half fires mid-attn (DMA-idle)
        for tt in range(NTT):
            proj_tt(1, tt)
        proj_vtrans(1)
        af_load(0, nc.sync)      # a2a0 done long ago
        for h0 in range(0, HPC, 2):
            attn_heads(1, h0)
            a2a_phase(1, h0 // 2)
        af_load(1, nc.gpsimd)    # queued right behind a2a1 -> no HOL block
        oproj_phase()

    nc.compile()
    return nc


def _prep_inputs(inputs):
    hidden = np.asarray(inputs["hidden_states"], np.float32)
    pos = np.asarray(inputs["position_ids"])
    cos = np.asarray(inputs["cos"], np.float32)
    sin = np.asarray(inputs["sin"], np.float32)
    wq = np.asarray(inputs["wq"], np.float32)
    wk = np.asarray(inputs["wk"], np.float32)
    wv = np.asarray(inputs["wv"], np.float32)
    wo = np.asarray(inputs["wo"], np.float32)
    qw = np.asarray(inputs["q_norm_w"], np.float32)
    kw = np.asarray(inputs["k_norm_w"], np.float32)

    # hT tiles: [B, FCH, NTT, 128, TT] contiguous per (b, f, tt)
    hTf = hidden.transpose(0, 2, 1).reshape(B, FCH, 128, NTT, TT)
    hT = np.ascontiguousarray(hTf.transpose(0, 1, 3, 2, 4)).astype(npbf16)
    cosT = np.ascontiguousarray(cos[pos].transpose(0, 2, 1)).astype(npbf16)
    sinT_f = sin[pos].transpose(0, 2, 1).copy()
    sinT_f[:, 0::2, :] *= -1.0
    sinT = np.ascontiguousarray(sinT_f).astype(npbf16)

    winvq = np.repeat(
        (1.0 / np.where(qw == 0, 1, qw) ** 2).reshape(D, 1), 128, axis=1
    ).astype(npbf16)
    winvk = np.repeat(
        (1.0 / np.where(kw == 0, 1, kw) ** 2).reshape(D, 1), 128, axis=1
    ).astype(npbf16)
    # wo: [NOB, 128, FCH, 512] (partition-major contiguous per ob block)
    woT = np.ascontiguousarray(
        wo.reshape(FCH, 128, NOB, 512).transpose(2, 1, 0, 3)
    ).astype(npbf16)

    def pmajor(w):
        # [HID, n] -> [128, FCH, n]
        n = w.shape[1]
        return np.ascontiguousarray(
            w.reshape(FCH, 128, n).transpose(1, 0, 2)
        ).astype(npbf16)

    in_maps = []
    for c in range(NC):
        wq_c = wq[:, c * QW:(c + 1) * QW].copy()
        for j in range(HPC):
            blk = wq_c[:, j * D:(j + 1) * D]
            blk -= blk.mean(axis=1, keepdims=True)
            blk *= qw[None, :]
        wk_c = wk[:, c * D:(c + 1) * D].copy()
        wk_c -= wk_c.mean(axis=1, keepdims=True)
        wk_c *= kw[None, :]
        in_maps.append({
            "hT": hT,
            "cosT": cosT,
            "sinT": sinT,
            "wq": pmajor(wq_c),
            "wk": pmajor(wk_c),
            "wv": pmajor(wv[:, c * D:(c + 1) * D]),
            "wo": woT,
            "winvq": winvq,
            "winvk": winvk,
        })
    return in_maps


def _run(inputs, **kwargs):
    if "nc" not in _CACHE:
        _CACHE["nc"] = _build_module()
    nc = _CACHE["nc"]
    in_maps = _prep_inputs(inputs)
    res = run_bass_kernel_spmd(nc, in_maps, core_ids=list(range(NC)), **kwargs)
    # core c holds out[b, :, :, :, :] for tokens c*TSL:(c+1)*TSL of each batch
    shards = []
    for c in range(NC):
        o = res.results[c]["out"].reshape(B, TSL // 128, NOB, 128, 512)
        # -> [B, TSL, HID]
        o = o.transpose(0, 1, 3, 2, 4).reshape(B, TSL, HID)
        shards.append(o)
    full = np.concatenate(shards, axis=1).astype(np.float32)
    return full, res


def kernel(**inputs) -> np.ndarray:
    out, _ = _run(inputs)
    return out


if __name__ == "__main__":
    import reference
    ins = {k: np.asarray(v) for k, v in reference.setup_inputs().items()}
    expected = np.asarray(reference.reference(**reference.setup_inputs()))
    actual = kernel(**ins)
    err = np.linalg.norm(actual - expected) / np.linalg.norm(expected)
    print("Relative error:", err)


# revision 12
# speedup vs baseline: 1.0486x; 1.0486x over previous
"""Trainium2 Bass kernel for CohereAttention (QK-LayerNorm + interleaved RoPE +
GQA sliding-window attention), sharded over 8 NeuronCores.

Sharding: tensor-parallel over Q heads (4 per core); with H//KVH == 4 each core
owns exactly one KV head. Attention outputs are exchanged with an AllToAll
(token-major blocks) and o_proj is token-parallel: each core computes the full
4096-wide o_proj output for its 256-token slice per batch, streaming wo ONCE
(shared across both batches).

Key engine/scheduling decisions (from NTFF trace analysis):
  - All matmuls contract over the partition axis; weights are host-retiled to
    partition-major contiguous blocks so every DMA moves large contiguous
    lines (the [c p]-interleaved layouts cost ~2x DMA efficiency and ~0.8us
    of issue time per dma_start).
  - DVE RECIPROCAL is ~16 cyc/elem on TRN2 -- all 1/x go through ACT Ln+Exp
    (or are split ACT/DVE in attention to balance engine load).
  - Partition broadcasts are free via matmul: stationary operands (winv, ones)
    are replicated to 128 identical columns, so the [128, T] result of the
    sum-of-squares / softmax-denominator matmul is already broadcast.
    gpsimd is left entirely to the collectives.
  - Attention processes two heads in a software pipeline (PV/ones lag the
    scores by one pair) so the PE never sits behind the ACT exp chain.
  - o_proj pairs output blocks (ob, ob+4) so each af stationary load feeds
    two 512-wide matmuls (12.5% LDWEIGHTS tax instead of 25%), streams wo
    once for both batches, and runs batch-0 chains first so the batch-1
    AllToAll completes in the shadow.
  - DMA queues: hT on sync, weights/wo on scalar, a2a writes/af/out on vector.
"""

import sys

sys.path.insert(0, "/opt/trn_rl_repo")

import numpy as np
import ml_dtypes

import concourse.bass as bass
import concourse.mybir as mybir
import concourse.tile as tile
from concourse import bacc
from concourse.bass import ts, ds
from concourse.bass_utils import run_bass_kernel_spmd

B, S, H, KVH, D, HID = 2, 2048, 32, 8, 128, 4096
WINDOW = 512
EPS = 1e-5
SCALE = float(D) ** -0.5
NC = 8
HPC = H // NC              # q heads per core (4)
QW = HPC * D               # q width per core (512)
FCH = HID // 128           # contraction chunks (32)
TT = 512                   # projection token tile
NTT = S // TT              # 4
QT = 256                   # attention query tile
NKC = (WINDOW + QT) // 128  # key chunks per query tile window (6)
TSL = S // NC              # tokens per (core, batch) slice for o_proj (256)
NOB = HID // 512           # o_proj output blocks (8)

BF16 = mybir.dt.bfloat16
F32 = mybir.dt.float32
npbf16 = ml_dtypes.bfloat16

SWAP32 = [i ^ 1 for i in range(32)]  # adjacent-pair partition swap

_CACHE = {}


def _edge_masks():
    jj = np.arange(128)[:, None]
    qi = np.arange(QT)[None, :]

    def m(off):
        u = off + qi - jj
        return ((u >= 0) & (u < WINDOW)).astype(npbf16)

    mw = np.concatenate([m(512), m(384)], axis=1)   # chunks kk=0,1 (window edge)
    mc = np.concatenate([m(0), m(-128)], axis=1)    # chunks kk=4,5 (causal edge)
    return mw, mc


def _build_module():
    nc = bacc.Bacc(
        "TRN2",
        target_bir_lowering=False,
        debug=False,
        enable_asserts=False,
        num_devices=NC,
    )

    # host-retiled inputs: everything partition-major / contiguous per DMA tile
    hT = nc.dram_tensor("hT", [B, FCH, NTT, 128, TT], BF16,
                        kind="ExternalInput").ap()
    cosT = nc.dram_tensor("cosT", [B, D, S], BF16, kind="ExternalInput").ap()
    sinT = nc.dram_tensor("sinT", [B, D, S], BF16, kind="ExternalInput").ap()
    wq = nc.dram_tensor("wq", [128, FCH, QW], BF16, kind="ExternalInput").ap()
    wk = nc.dram_tensor("wk", [128, FCH, D], BF16, kind="ExternalInput").ap()
    wv = nc.dram_tensor("wv", [128, FCH, D], BF16, kind="ExternalInput").ap()
    wo = nc.dram_tensor("wo", [NOB, 128, FCH, 512], BF16,
                        kind="ExternalInput").ap()
    winvq = nc.dram_tensor("winvq", [D, 128], BF16, kind="ExternalInput").ap()
    winvk = nc.dram_tensor("winvk", [D, 128], BF16, kind="ExternalInput").ap()
    out = nc.dram_tensor("out", [B, TSL // 128, NOB, 128, 512], BF16,
                         kind="ExternalOutput").ap()

    a2ain = [
        nc.dram_tensor(f"a2ain{b}", [NC, HPC, 128, TSL], BF16,
                       kind="Internal").ap()
        for b in range(B)
    ]
    a2aout = [
        nc.dram_tensor(f"a2aout{b}", [NC, HPC, 128, TSL], BF16,
                       kind="Internal").ap()
        for b in range(B)
    ]

    ones_d = nc.inline_tensor(np.ones((128, 128), dtype=npbf16),
                              name="ones128").ap()
    mw_np, mc_np = _edge_masks()
    maskw_d = nc.inline_tensor(mw_np, name="maskw").ap()
    maskc_d = nc.inline_tensor(mc_np, name="maskc").ap()

    rg = [list(range(NC))]

    with tile.TileContext(nc) as tc, \
            tc.tile_pool(name="sb", bufs=1) as sb, \
            tc.tile_pool(name="ps", bufs=1, space="PSUM") as ps:

        # --- resident weights / constants ---
        # First f-blocks of wq/wk/wv ride the Sync queue so the first
        # projection matmuls start within a few us; everything else goes on
        # the Scalar queue so it never blocks the streamed hT tiles.
        wq_sb = sb.tile([128, FCH, QW], BF16, tag="wq", bufs=1, name="wq_sb")
        wk_sb = sb.tile([128, FCH, D], BF16, tag="wk", bufs=1, name="wk_sb")
        wv_sb = sb.tile([128, FCH, D], BF16, tag="wv", bufs=1, name="wv_sb")
        nc.sync.dma_start(wq_sb[:, ds(0, 4), :], wq[:, ds(0, 4), :])
        nc.sync.dma_start(wk_sb[:, ds(0, 4), :], wk[:, ds(0, 4), :])
        nc.sync.dma_start(wv_sb[:, ds(0, 4), :], wv[:, ds(0, 4), :])
        # f4..31 stream inline with the first token tile (need-order) so the
        # PE never waits ~20us for the full weight preload
        ones_sb = sb.tile([128, 128], BF16, tag="ones", bufs=1, name="ones_sb")
        nc.scalar.dma_start(ones_sb[:], ones_d)
        maskw_sb = sb.tile([128, 2 * QT], BF16, tag="maskw", bufs=1,
                           name="maskw_sb")
        nc.scalar.dma_start(maskw_sb[:], maskw_d)
        maskc_sb = sb.tile([128, 2 * QT], BF16, tag="maskc", bufs=1,
                           name="maskc_sb")
        nc.scalar.dma_start(maskc_sb[:], maskc_d)
        winvq_sb = sb.tile([D, 128], BF16, tag="winvq", bufs=1, name="winvq_sb")
        nc.scalar.dma_start(winvq_sb[:], winvq)
        winvk_sb = sb.tile([D, 128], BF16, tag="winvk", bufs=1, name="winvk_sb")
        nc.scalar.dma_start(winvk_sb[:], winvk)
        eps_sb = sb.tile([128, 1], F32, tag="eps", bufs=1, name="eps_sb")
        nc.vector.memset(eps_sb[:], EPS)

        def ln_rope(qsb, winv_sb, cos_sb, sin_sb, tt, dst, sn):
            """LayerNorm (mean pre-folded on host) + interleaved RoPE on a
            drained [d, TT] bf16 tile; writes bf16 into dst[:, tt*TT:...].

            rstd = exp(-0.5*ln(ssq/D + eps)) on ACT -- DVE reciprocal is
            ~16 cyc/elem, Ln+Exp are ~1 cyc/col.  ssq comes out of the PE
            already broadcast to 128 partitions because winv_sb has 128
            identical columns (matmul cost only depends on the free size)."""
            sq = sb.tile([128, TT], BF16, tag="sq", bufs=2, name="sq")
            nc.vector.tensor_mul(sq[:], qsb[:], qsb[:])
            ssq = ps.tile([128, TT], F32, tag=f"s{sn}", bufs=1, name="ssq")
            nc.tensor.matmul(ssq[:], winv_sb[:], sq[:], start=True, stop=True)
            lt = sb.tile([128, TT], F32, tag="lt", bufs=2, name="lt")
            nc.scalar.activation(
                lt[:], ssq[:], mybir.ActivationFunctionType.Ln,
                bias=eps_sb[:], scale=1.0 / D,
            )
            rstd = sb.tile([128, TT], BF16, tag="rstd", bufs=2, name="rstd")
            nc.scalar.activation(
                rstd[:], lt[:], mybir.ActivationFunctionType.Exp, scale=-0.5,
            )
            qn = sb.tile([128, TT], BF16, tag="qn", bufs=2, name="qn")
            nc.vector.tensor_mul(qn[:], qsb[:], rstd[:])
            qs = sb.tile([128, TT], BF16, tag="qs", bufs=2, name="qs")
            nc.vector.stream_shuffle(qs[:], qn[:], SWAP32)
            t1 = sb.tile([128, TT], BF16, tag="t1", bufs=2, name="t1")
            nc.vector.tensor_mul(t1[:], qn[:], cos_sb[:, ts(tt, TT)])
            t2 = sb.tile([128, TT], BF16, tag="t2", bufs=2, name="t2")
            nc.vector.tensor_mul(t2[:], qs[:], sin_sb[:, ts(tt, TT)])
            nc.vector.tensor_add(dst[:, ts(tt, TT)], t1[:], t2[:])

        qT = {}   # (b, h) -> [128, S] bf16 rope'd normalized q, transposed
        kT = {}   # b -> [128, S]
        Vn = {}   # b -> [128, S//128, 128] natural [j, d] chunks
        vT = {}   # b -> [128, S] transposed v (pre transpose)
        trig = {}  # b -> (cos_sb, sin_sb)

        def proj_setup(b):
            cos_sb = sb.tile([128, S], BF16, tag="cos", bufs=1, name="cos_sb")
            nc.scalar.dma_start(cos_sb[:], cosT[b])
            sin_sb = sb.tile([128, S], BF16, tag="sin", bufs=1, name="sin_sb")
            nc.scalar.dma_start(sin_sb[:], sinT[b])
            trig[b] = (cos_sb, sin_sb)
            for h in range(HPC):
                qT[(b, h)] = sb.tile([128, S], BF16, tag="qT", bufs=4,
                                     name=f"qT{b}{h}")
            kT[b] = sb.tile([128, S], BF16, tag="kT", bufs=2, name=f"kT{b}")
            vT[b] = sb.tile([128, S], BF16, tag="vT", bufs=2, name=f"vT{b}")
            Vn[b] = sb.tile([128, S // 128, 128], BF16, tag="Vn", bufs=2,
                            name=f"Vn{b}")

        def proj_tt(b, tt):
            with nc.named_scope(f"proj_b{b}"):
                cos_sb, sin_sb = trig[b]
                qps = [
                    ps.tile([128, TT], F32, tag=f"acc{i}", bufs=1,
                            name=f"qps{i}")
                    for i in range(HPC)
                ]
                kps = ps.tile([128, TT], F32, tag="acck", bufs=1, name="kps")
                vps = ps.tile([128, TT], F32, tag="accv", bufs=1, name="vps")
                for f in range(FCH):
                    if b == 0 and tt == 0 and f >= 4:
                        # need-order weight streaming, alternating queues
                        eng = nc.sync if f % 2 == 0 else nc.scalar
                        eng.dma_start(wq_sb[:, f, :], wq[:, f, :])
                        eng.dma_start(wk_sb[:, f, :], wk[:, f, :])
                        eng.dma_start(wv_sb[:, f, :], wv[:, f, :])
                    ht_t = sb.tile([128, TT], BF16, tag="ht", bufs=7,
                                   name="ht_t")
                    if f % 2 == 0:
                        nc.sync.dma_start(ht_t[:], hT[b, f, tt])
                    else:
                        nc.scalar.dma_start(ht_t[:], hT[b, f, tt])
                    st = f == 0
                    sp = f == FCH - 1
                    for h in range(HPC):
                        nc.tensor.matmul(
                            qps[h][:], wq_sb[:, f, ts(h, D)], ht_t[:],
                            start=st, stop=sp,
                        )
                    nc.tensor.matmul(kps[:], wk_sb[:, f, :], ht_t[:],
                                     start=st, stop=sp)
                    nc.tensor.matmul(vps[:], wv_sb[:, f, :], ht_t[:],
                                     start=st, stop=sp)
                # Drain all six PSUM banks, alternating engines, so the next
                # tile's accumulation chains unblock as early as possible.
                qsb = []
                for i in range(HPC):
                    q = sb.tile([128, TT], BF16, tag="qsb", bufs=4,
                                name=f"qsb{i}")
                    if i % 2 == 0:
                        nc.scalar.copy(q[:], qps[i][:])
                    else:
                        nc.vector.tensor_copy(q[:], qps[i][:])
                    qsb.append(q)
                ksb = sb.tile([128, TT], BF16, tag="qsb", bufs=4, name="ksb")
                nc.scalar.copy(ksb[:], kps[:])
                nc.vector.tensor_copy(vT[b][:, ts(tt, TT)], vps[:])
                for h in range(HPC):
                    ln_rope(qsb[h], winvq_sb, cos_sb, sin_sb, tt, qT[(b, h)],
                            h % 2)
                ln_rope(ksb, winvk_sb, cos_sb, sin_sb, tt, kT[b], 0)

        def proj_vtrans(b):
            # transpose v to natural [j, d] chunk layout via the DMA xbar
            with nc.named_scope(f"proj_b{b}"):
                nc.scalar.dma_start_transpose(Vn[b][:], vT[b][:])

        def attn_heads(b, h0):
            """Two heads (h0, h0+1) interleaved; PV/ones lag scores by one
            pair so the PE stream never waits on the ACT exp."""
            with nc.named_scope(f"attn_b{b}"):
                streams = []
                for si, h in enumerate((h0, h0 + 1)):
                    attn_sb = sb.tile([128, S], BF16, tag=f"attn{si}", bufs=2,
                                      name=f"attn_sb{si}")
                    streams.append({"h": h, "si": si, "attn": attn_sb})

                def scores(st, qt, pp, kstart):
                    si, h = st["si"], st["h"]
                    i0 = qt * QT
                    kk = kstart + 2 * pp
                    j0 = i0 - WINDOW + kk * 128
                    sps = ps.tile([128, 2 * QT], F32,
                                  tag=f"acc{2 * si + pp % 2}", bufs=1,
                                  name="sps")
                    nc.tensor.matmul(
                        sps[:, 0:QT], kT[b][:, ds(j0, 128)],
                        qT[(b, h)][:, ds(i0, QT)],
                        start=True, stop=True,
                    )
                    nc.tensor.matmul(
                        sps[:, QT:2 * QT], kT[b][:, ds(j0 + 128, 128)],
                        qT[(b, h)][:, ds(i0, QT)],
                        start=True, stop=True,
                    )
                    pt = sb.tile([128, 2 * QT], BF16, tag=f"pt{si}", bufs=2,
                                 name="pt")
                    nc.scalar.activation(
                        pt[:], sps[:], mybir.ActivationFunctionType.Exp,
                        scale=SCALE,
                    )
                    if kk == 0:      # window edge pair (kk=0,1)
                        nc.vector.tensor_mul(pt[:], pt[:], maskw_sb[:])
                    elif kk == 4:    # causal edge pair (kk=4,5)
                        nc.vector.tensor_mul(pt[:], pt[:], maskc_sb[:])
                    return pt

                def pv(st, qt, pp, kstart, npair, pt, ops, lps):
                    i0 = qt * QT
                    kk = kstart + 2 * pp
                    j0 = i0 - WINDOW + kk * 128
                    first = pp == 0
                    last = pp == npair - 1
                    jc = j0 // 128
                    nc.tensor.matmul(
                        ops[:], Vn[b][:, jc, :], pt[:, 0:QT],
                        start=first, stop=False,
                    )
                    nc.tensor.matmul(
                        ops[:], Vn[b][:, jc + 1, :], pt[:, QT:2 * QT],
                        start=False, stop=last,
                    )
                    # softmax denominator, broadcast to all 128 partitions by
                    # the 128-identical-column ones stationary
                    nc.tensor.matmul(lps[:], ones_sb[:], pt[:, 0:QT],
                                     start=first, stop=False)
                    nc.tensor.matmul(lps[:], ones_sb[:], pt[:, QT:2 * QT],
                                     start=False, stop=last)

                def finish_qt(st, qt, ops, lps):
                    # Drain ops to SBUF immediately (frees the PSUM bank so
                    # the next tile's PV chain starts without waiting for the
                    # denominator), then normalize from SBUF.
                    si = st["si"]
                    i0 = qt * QT
                    osb = sb.tile([128, QT], F32, tag=f"osb{si}", bufs=2,
                                  name="osb")
                    nc.vector.tensor_copy(osb[:], ops[:])
                    linv = sb.tile([128, QT], F32, tag=f"linv{si}", bufs=2,
                                   name="linv")
                    if si == 0:
                        # ACT: linv = exp(-ln(l))
                        lt = sb.tile([128, QT], F32, tag="alt", bufs=2,
                                     name="alt")
                        nc.scalar.activation(
                            lt[:], lps[:], mybir.ActivationFunctionType.Ln,
                        )
                        nc.scalar.activation(
                            linv[:], lt[:], mybir.ActivationFunctionType.Exp,
                            scale=-1.0,
                        )
                    else:
                        # DVE reciprocal for the other stream (engine balance)
                        nc.vector.reciprocal(linv[:], lps[:])
                    nc.vector.tensor_mul(st["attn"][:, ds(i0, QT)], osb[:],
                                         linv[:])

                for qt in range(S // QT):
                    i0 = qt * QT
                    kstart = max(0, (WINDOW - i0) // 128)
                    npair = (NKC - kstart) // 2
                    ops = {}
                    lps = {}
                    for st in streams:
                        si = st["si"]
                        ops[si] = ps.tile([128, QT], F32,
                                          tag="acck" if si == 0 else "accv",
                                          bufs=1, name="ops")
                        lps[si] = ps.tile([128, QT], F32,
                                          tag="s0" if si == 0 else "s1",
                                          bufs=1, name="lps")
                    pts = {0: {}, 1: {}}
                    for pp in range(npair):
                        for st in streams:
                            pts[st["si"]][pp] = scores(st, qt, pp, kstart)
                        if pp > 0:
                            for st in streams:
                                si = st["si"]
                                pv(st, qt, pp - 1, kstart, npair,
                                   pts[si][pp - 1], ops[si], lps[si])
                                del pts[si][pp - 1]
                    for st in streams:
                        si = st["si"]
                        pv(st, qt, npair - 1, kstart, npair,
                           pts[si][npair - 1], ops[si], lps[si])
                    for st in streams:
                        finish_qt(st, qt, ops[st["si"]], lps[st["si"]])
                # ship both heads' outputs: one contiguous 64KB DMA per
                # (head, peer) on the gpsimd queue -- the collective that
                # consumes them lives there too, so ordering is natural and
                # the sync/scalar queues stay free for the next phase's
                # prefetches.
                for st in streams:
                    for n in range(NC):
                        nc.gpsimd.dma_start(
                            a2ain[b][n, st["h"]],
                            st["attn"][:, ts(n, TSL)],
                        )

        def a2a_phase(b):
            nc.gpsimd.collective_compute(
                "AllToAll",
                mybir.AluOpType.bypass,
                replica_groups=rg,
                ins=[a2ain[b][:]],
                outs=[a2aout[b][:]],
            )

        af = {}

        def af_load(b, eng):
            af[b] = sb.tile([128, NC, HPC, TSL], BF16, tag=f"af{b}", bufs=1,
                            name=f"af{b}")
            eng.dma_start(
                af[b][:], a2aout[b].rearrange("n c p t -> p n c t"),
            )

        PTAGS = ["acc0", "acc1", "acc2", "acc3", "acck", "accv", "s0", "s1"]

        def oproj_wo_load(obp):
            oblo, obhi = obp, obp + NOB // 2
            wo_lo = []
            wo_hi = []
            for i in range(FCH // 2):
                wl = sb.tile([128, 2, 512], BF16, tag="wolo", bufs=3,
                             name="wo_lo")
                wh = sb.tile([128, 2, 512], BF16, tag="wohi", bufs=3,
                             name="wo_hi")
                if i % 2 == 0:
                    nc.scalar.dma_start(wl[:], wo[oblo][:, ds(2 * i, 2), :])
                    nc.sync.dma_start(wh[:], wo[obhi][:, ds(2 * i, 2), :])
                else:
                    nc.sync.dma_start(wl[:], wo[oblo][:, ds(2 * i, 2), :])
                    nc.scalar.dma_start(wh[:], wo[obhi][:, ds(2 * i, 2), :])
                wo_lo.append(wl)
                wo_hi.append(wh)
            return wo_lo, wo_hi

        def oproj_obp(obp, groups, wo_pre=None):
            # one output-block pair (ob, ob+4) of o_proj for the given
            # (batch, t2) groups.  wo streams in contiguous 2-chunk pieces
            # alternating scalar/sync; each af stationary chunk feeds two
            # 512-wide matmuls (halved LDWEIGHTS tax).  c-outer piece order
            # means pieces are consumed sequentially -> small bufs stream.
            oblo, obhi = obp, obp + NOB // 2
            wo_lo, wo_hi = wo_pre if wo_pre else oproj_wo_load(obp)
            po = {}
            for gi, (bb, t2) in enumerate(groups):
                po[(bb, t2)] = (
                    ps.tile([128, 512], F32, tag=PTAGS[2 * gi], bufs=1,
                            name="plo"),
                    ps.tile([128, 512], F32, tag=PTAGS[2 * gi + 1], bufs=1,
                            name="phi"),
                )
            for pc in range(FCH // 2):
                for cc in range(2):
                    c = 2 * pc + cc
                    st = c == 0
                    sp = c == FCH - 1
                    for bb, t2 in groups:
                        stat = af[bb][:, c // HPC, c % HPC, ds(t2 * 128, 128)]
                        p_lo, p_hi = po[(bb, t2)]
                        nc.tensor.matmul(p_lo[:], stat, wo_lo[pc][:, cc, :],
                                         start=st, stop=sp)
                        nc.tensor.matmul(p_hi[:], stat, wo_hi[pc][:, cc, :],
                                         start=st, stop=sp)
            for gi, (bb, t2) in enumerate(groups):
                p_lo, p_hi = po[(bb, t2)]
                for ob, pp in ((oblo, p_lo), (obhi, p_hi)):
                    ot = sb.tile([128, 512], BF16, tag="ot", bufs=2,
                                 name="ot")
                    if (t2 + ob) % 2 == 0:
                        nc.scalar.copy(ot[:], pp[:])
                    else:
                        nc.vector.tensor_copy(ot[:], pp[:])
                    if ob % 2 == 0:
                        nc.sync.dma_start(out[bb, t2, ob], ot[:])
                    else:
                        nc.scalar.dma_start(out[bb, t2, ob], ot[:])

        def oproj_phase(wo_x):
            with nc.named_scope("oproj"):
                # batch-0-only pre-pass on obp0 (wo prefetched during attn1)
                # hides the batch-1 AllToAll; then the fused remainder
                # streams wo once for both batches
                oproj_obp(0, [(0, 0), (0, 1)], wo_pre=wo_x)
                oproj_obp(0, [(1, 0), (1, 1)])
                for obp in range(1, NOB // 2):
                    oproj_obp(obp, [(0, 0), (0, 1), (1, 0), (1, 1)])

        proj_setup(0)
        for tt in range(NTT):
            proj_tt(0, tt)
        proj_vtrans(0)
        proj_setup(1)    # cos/sin for batch 1 load during attn0
        for h0 in range(0, HPC, 2):
            attn_heads(0, h0)
        a2a_phase(0)
        for tt in range(NTT):
            proj_tt(1, tt)
        proj_vtrans(1)
        af_load(0, nc.sync)      # a2a0 done long ago
        wo_x = oproj_wo_load(0)  # X-pass wo streams during attn1 (idle DMA)
        for h0 in range(0, HPC, 2):
            attn_heads(1, h0)
        a2a_phase(1)
        af_load(1, nc.gpsimd)    # queued right behind a2a1 -> no HOL block
        oproj_phase(wo_x)

    nc.compile()
    return nc


def _prep_inputs(inputs):
    hidden = np.asarray(inputs["hidden_states"], np.float32)
    pos = np.asarray(inputs["position_ids"])
    cos = np.asarray(inputs["cos"], np.float32)
    sin = np.asarray(inputs["sin"], np.float32)
    wq = np.asarray(inputs["wq"], np.float32)
    wk = np.asarray(inputs["wk"], np.float32)
    wv = np.asarray(inputs["wv"], np.float32)
    wo = np.asarray(inputs["wo"], np.float32)
    qw = np.asarray(inputs["q_norm_w"], np.float32)
    kw = np.asarray(inputs["k_norm_w"], np.float32)

    # hT tiles: [B, FCH, NTT, 128, TT] contiguous per (b, f, tt)
    hTf = hidden.transpose(0, 2, 1).reshape(B, FCH, 128, NTT, TT)
    hT = np.ascontiguousarray(hTf.transpose(0, 1, 3, 2, 4)).astype(npbf16)
    cosT = np.ascontiguousarray(cos[pos].transpose(0, 2, 1)).astype(npbf16)
    sinT_f = sin[pos].transpose(0, 2, 1).copy()
    sinT_f[:, 0::2, :] *= -1.0
    sinT = np.ascontiguousarray(sinT_f).astype(npbf16)

    winvq = np.repeat(
        (1.0 / np.where(qw == 0, 1, qw) ** 2).reshape(D, 1), 128, axis=1
    ).astype(npbf16)
    winvk = np.repeat(
        (1.0 / np.where(kw == 0, 1, kw) ** 2).reshape(D, 1), 128, axis=1
    ).astype(npbf16)
    # wo: [NOB, 128, FCH, 512] (partition-major contiguous per ob block)
    woT = np.ascontiguousarray(
        wo.reshape(FCH, 128, NOB, 512).transpose(2, 1, 0, 3)
    ).astype(npbf16)

    def pmajor(w):
        # [HID, n] -> [128, FCH, n]
        n = w.shape[1]
        return np.ascontiguousarray(
            w.reshape(FCH, 128, n).transpose(1, 0, 2)
        ).astype(npbf16)

    in_maps = []
    for c in range(NC):
        wq_c = wq[:, c * QW:(c + 1) * QW].copy()
        for j in range(HPC):
            blk = wq_c[:, j * D:(j + 1) * D]
            blk -= blk.mean(axis=1, keepdims=True)
            blk *= qw[None, :]
        wk_c = wk[:, c * D:(c + 1) * D].copy()
        wk_c -= wk_c.mean(axis=1, keepdims=True)
        wk_c *= kw[None, :]
        in_maps.append({
            "hT": hT,
            "cosT": cosT,
            "sinT": sinT,
            "wq": pmajor(wq_c),
            "wk": pmajor(wk_c),
            "wv": pmajor(wv[:, c * D:(c + 1) * D]),
            "wo": woT,
            "winvq": winvq,
            "winvk": winvk,
        })
    return in_maps


def _run(inputs, **kwargs):
    if "nc" not in _CACHE:
        _CACHE["nc"] = _build_module()
    nc = _CACHE["nc"]
    in_maps = _prep_inputs(inputs)
    res = run_bass_kernel_spmd(nc, in_maps, core_ids=list(range(NC)), **kwargs)
    # core c holds out[b, :, :, :, :] for tokens c*TSL:(c+1)*TSL of each batch
    shards = []
    for c in range(NC):
        o = res.results[c]["out"].reshape(B, TSL // 128, NOB, 128, 512)
        # -> [B, TSL, HID]
        o = o.transpose(0, 1, 3, 2, 4).reshape(B, TSL, HID)
        shards.append(o)
    full = np.concatenate(shards, axis=1).astype(np.float32)
    return full, res


def kernel(**inputs) -> np.ndarray:
    out, _ = _run(inputs)
    return out


if __name__ == "__main__":
    import reference
    ins = {k: np.asarray(v) for k, v in reference.setup_inputs().items()}
    expected = np.asarray(reference.reference(**reference.setup_inputs()))
    actual = kernel(**ins)
    err = np.linalg.norm(actual - expected) / np.linalg.norm(expected)
    print("Relative error:", err)


# revision 13
# speedup vs baseline: 1.1313x; 1.0789x over previous
"""Trainium2 Bass kernel for CohereAttention (QK-LayerNorm + interleaved RoPE +
GQA sliding-window attention), sharded over 8 NeuronCores.

Sharding: tensor-parallel over Q heads (4 per core); with H//KVH == 4 each core
owns exactly one KV head. Attention outputs are exchanged with an AllToAll
(token-major blocks) and o_proj is token-parallel: each core computes the full
4096-wide o_proj output for its 256-token slice per batch, streaming wo ONCE
(shared across both batches).

Key engine/scheduling decisions (from NTFF trace analysis):
  - All matmuls contract over the partition axis; weights are host-retiled to
    partition-major contiguous blocks so every DMA moves large contiguous
    lines (the [c p]-interleaved layouts cost ~2x DMA efficiency and ~0.8us
    of issue time per dma_start).
  - DVE RECIPROCAL is ~16 cyc/elem on TRN2 -- all 1/x go through ACT Ln+Exp
    (or are split ACT/DVE in attention to balance engine load).
  - Partition broadcasts are free via matmul: stationary operands (winv, ones)
    are replicated to 128 identical columns, so the [128, T] result of the
    sum-of-squares / softmax-denominator matmul is already broadcast.
    gpsimd is left entirely to the collectives.
  - Attention processes two heads in a software pipeline (PV/ones lag the
    scores by one pair) so the PE never sits behind the ACT exp chain.
  - o_proj pairs output blocks (ob, ob+4) so each af stationary load feeds
    two 512-wide matmuls (12.5% LDWEIGHTS tax instead of 25%), streams wo
    once for both batches, and runs batch-0 chains first so the batch-1
    AllToAll completes in the shadow.
  - DMA queues: hT on sync, weights/wo on scalar, a2a writes/af/out on vector.
"""

import sys

sys.path.insert(0, "/opt/trn_rl_repo")

import numpy as np
import ml_dtypes

import concourse.bass as bass
import concourse.mybir as mybir
import concourse.tile as tile
from concourse import bacc
from concourse.bass import ts, ds
from concourse.bass_utils import run_bass_kernel_spmd

B, S, H, KVH, D, HID = 2, 2048, 32, 8, 128, 4096
WINDOW = 512
EPS = 1e-5
SCALE = float(D) ** -0.5
NC = 8
HPC = H // NC              # q heads per core (4)
QW = HPC * D               # q width per core (512)
FCH = HID // 128           # contraction chunks (32)
TT = 512                   # projection token tile
NTT = S // TT              # 4
QT = 256                   # attention query tile
NKC = (WINDOW + QT) // 128  # key chunks per query tile window (6)
TSL = S // NC              # tokens per (core, batch) slice for o_proj (256)
NOB = HID // 512           # o_proj output blocks (8)

BF16 = mybir.dt.bfloat16
F32 = mybir.dt.float32
npbf16 = ml_dtypes.bfloat16

SWAP32 = [i ^ 1 for i in range(32)]  # adjacent-pair partition swap

_CACHE = {}


def _edge_masks():
    jj = np.arange(128)[:, None]
    qi = np.arange(QT)[None, :]

    def m(off):
        u = off + qi - jj
        return ((u >= 0) & (u < WINDOW)).astype(npbf16)

    mw = np.concatenate([m(512), m(384)], axis=1)   # chunks kk=0,1 (window edge)
    mc = np.concatenate([m(0), m(-128)], axis=1)    # chunks kk=4,5 (causal edge)
    return mw, mc


def _build_module():
    nc = bacc.Bacc(
        "TRN2",
        target_bir_lowering=False,
        debug=False,
        enable_asserts=False,
        num_devices=NC,
    )

    # host-retiled inputs: everything partition-major / contiguous per DMA tile
    hT = nc.dram_tensor("hT", [B, FCH, NTT, 128, TT], BF16,
                        kind="ExternalInput").ap()
    cosT = nc.dram_tensor("cosT", [B, D, S], BF16, kind="ExternalInput").ap()
    sinT = nc.dram_tensor("sinT", [B, D, S], BF16, kind="ExternalInput").ap()
    wq = nc.dram_tensor("wq", [128, FCH, QW], BF16, kind="ExternalInput").ap()
    wk = nc.dram_tensor("wk", [128, FCH, D], BF16, kind="ExternalInput").ap()
    wv = nc.dram_tensor("wv", [128, FCH, D], BF16, kind="ExternalInput").ap()
    wo = nc.dram_tensor("wo", [NOB, 128, FCH, 512], BF16,
                        kind="ExternalInput").ap()
    winvq = nc.dram_tensor("winvq", [D, 128], BF16, kind="ExternalInput").ap()
    winvk = nc.dram_tensor("winvk", [D, 128], BF16, kind="ExternalInput").ap()
    out = nc.dram_tensor("out", [B, TSL // 128, NOB, 128, 512], BF16,
                         kind="ExternalOutput").ap()

    a2ain = [
        nc.dram_tensor(f"a2ain{b}", [NC, HPC, 128, TSL], BF16,
                       kind="Internal").ap()
        for b in range(B)
    ]
    a2aout = [
        nc.dram_tensor(f"a2aout{b}", [NC, HPC, 128, TSL], BF16,
                       kind="Internal").ap()
        for b in range(B)
    ]

    ones_d = nc.inline_tensor(np.ones((128, 128), dtype=npbf16),
                              name="ones128").ap()
    mw_np, mc_np = _edge_masks()
    maskw_d = nc.inline_tensor(mw_np, name="maskw").ap()
    maskc_d = nc.inline_tensor(mc_np, name="maskc").ap()

    rg = [list(range(NC))]

    with tile.TileContext(nc) as tc, \
            tc.tile_pool(name="sb", bufs=1) as sb, \
            tc.tile_pool(name="ps", bufs=1, space="PSUM") as ps:

        # --- resident weights / constants ---
        # First f-blocks of wq/wk/wv ride the Sync queue so the first
        # projection matmuls start within a few us; everything else goes on
        # the Scalar queue so it never blocks the streamed hT tiles.
        wq_sb = sb.tile([128, FCH, QW], BF16, tag="wq", bufs=1, name="wq_sb")
        wk_sb = sb.tile([128, FCH, D], BF16, tag="wk", bufs=1, name="wk_sb")
        wv_sb = sb.tile([128, FCH, D], BF16, tag="wv", bufs=1, name="wv_sb")
        nc.sync.dma_start(wq_sb[:, ds(0, 4), :], wq[:, ds(0, 4), :])
        nc.sync.dma_start(wk_sb[:, ds(0, 4), :], wk[:, ds(0, 4), :])
        nc.sync.dma_start(wv_sb[:, ds(0, 4), :], wv[:, ds(0, 4), :])
        # f4..31 stream inline with the first token tile (need-order) so the
        # PE never waits ~20us for the full weight preload
        ones_sb = sb.tile([128, 128], BF16, tag="ones", bufs=1, name="ones_sb")
        nc.scalar.dma_start(ones_sb[:], ones_d)
        maskw_sb = sb.tile([128, 2 * QT], BF16, tag="maskw", bufs=1,
                           name="maskw_sb")
        nc.scalar.dma_start(maskw_sb[:], maskw_d)
        maskc_sb = sb.tile([128, 2 * QT], BF16, tag="maskc", bufs=1,
                           name="maskc_sb")
        nc.scalar.dma_start(maskc_sb[:], maskc_d)
        winvq_sb = sb.tile([D, 128], BF16, tag="winvq", bufs=1, name="winvq_sb")
        nc.scalar.dma_start(winvq_sb[:], winvq)
        winvk_sb = sb.tile([D, 128], BF16, tag="winvk", bufs=1, name="winvk_sb")
        nc.scalar.dma_start(winvk_sb[:], winvk)
        eps_sb = sb.tile([128, 1], F32, tag="eps", bufs=1, name="eps_sb")
        nc.vector.memset(eps_sb[:], EPS)

        def ln_rope(qsb, winv_sb, cos_sb, sin_sb, tt, dst, sn):
            """LayerNorm (mean pre-folded on host) + interleaved RoPE on a
            drained [d, TT] bf16 tile; writes bf16 into dst[:, tt*TT:...].

            rstd = exp(-0.5*ln(ssq/D + eps)) on ACT -- DVE reciprocal is
            ~16 cyc/elem, Ln+Exp are ~1 cyc/col.  ssq comes out of the PE
            already broadcast to 128 partitions because winv_sb has 128
            identical columns (matmul cost only depends on the free size)."""
            sq = sb.tile([128, TT], BF16, tag="sq", bufs=2, name="sq")
            nc.vector.tensor_mul(sq[:], qsb[:], qsb[:])
            ssq = ps.tile([128, TT], F32, tag=f"s{sn}", bufs=1, name="ssq")
            nc.tensor.matmul(ssq[:], winv_sb[:], sq[:], start=True, stop=True)
            lt = sb.tile([128, TT], F32, tag="lt", bufs=2, name="lt")
            nc.scalar.activation(
                lt[:], ssq[:], mybir.ActivationFunctionType.Ln,
                bias=eps_sb[:], scale=1.0 / D,
            )
            rstd = sb.tile([128, TT], BF16, tag="rstd", bufs=2, name="rstd")
            nc.scalar.activation(
                rstd[:], lt[:], mybir.ActivationFunctionType.Exp, scale=-0.5,
            )
            qn = sb.tile([128, TT], BF16, tag="qn", bufs=2, name="qn")
            nc.vector.tensor_mul(qn[:], qsb[:], rstd[:])
            qs = sb.tile([128, TT], BF16, tag="qs", bufs=2, name="qs")
            nc.vector.stream_shuffle(qs[:], qn[:], SWAP32)
            t1 = sb.tile([128, TT], BF16, tag="t1", bufs=2, name="t1")
            nc.vector.tensor_mul(t1[:], qn[:], cos_sb[:, ts(tt, TT)])
            t2 = sb.tile([128, TT], BF16, tag="t2", bufs=2, name="t2")
            nc.vector.tensor_mul(t2[:], qs[:], sin_sb[:, ts(tt, TT)])
            nc.vector.tensor_add(dst[:, ts(tt, TT)], t1[:], t2[:])

        qT = {}   # (b, h) -> [128, S] bf16 rope'd normalized q, transposed
        kT = {}   # b -> [128, S]
        Vn = {}   # b -> [128, S//128, 128] natural [j, d] chunks
        vT = {}   # b -> [128, S] transposed v (pre transpose)
        trig = {}  # b -> (cos_sb, sin_sb)

        def proj_setup(b):
            cos_sb = sb.tile([128, S], BF16, tag="cos", bufs=1, name="cos_sb")
            nc.scalar.dma_start(cos_sb[:], cosT[b])
            sin_sb = sb.tile([128, S], BF16, tag="sin", bufs=1, name="sin_sb")
            nc.scalar.dma_start(sin_sb[:], sinT[b])
            trig[b] = (cos_sb, sin_sb)
            for h in range(HPC):
                qT[(b, h)] = sb.tile([128, S], BF16, tag="qT", bufs=4,
                                     name=f"qT{b}{h}")
            kT[b] = sb.tile([128, S], BF16, tag="kT", bufs=2, name=f"kT{b}")
            vT[b] = sb.tile([128, S], BF16, tag="vT", bufs=2, name=f"vT{b}")
            Vn[b] = sb.tile([128, S // 128, 128], BF16, tag="Vn", bufs=2,
                            name=f"Vn{b}")

        def proj_tt(b, tt):
            with nc.named_scope(f"proj_b{b}"):
                cos_sb, sin_sb = trig[b]
                qps = [
                    ps.tile([128, TT], F32, tag=f"acc{i}", bufs=1,
                            name=f"qps{i}")
                    for i in range(HPC)
                ]
                kps = ps.tile([128, TT], F32, tag="acck", bufs=1, name="kps")
                vps = ps.tile([128, TT], F32, tag="accv", bufs=1, name="vps")
                for f in range(FCH):
                    if b == 0 and tt == 0 and f >= 4:
                        # need-order weight streaming, alternating queues
                        eng = nc.sync if f % 2 == 0 else nc.scalar
                        eng.dma_start(wq_sb[:, f, :], wq[:, f, :])
                        eng.dma_start(wk_sb[:, f, :], wk[:, f, :])
                        eng.dma_start(wv_sb[:, f, :], wv[:, f, :])
                    ht_t = sb.tile([128, TT], BF16, tag="ht", bufs=9,
                                   name="ht_t")
                    if f % 2 == 0:
                        nc.sync.dma_start(ht_t[:], hT[b, f, tt])
                    else:
                        nc.scalar.dma_start(ht_t[:], hT[b, f, tt])
                    st = f == 0
                    sp = f == FCH - 1
                    for h in range(HPC):
                        nc.tensor.matmul(
                            qps[h][:], wq_sb[:, f, ts(h, D)], ht_t[:],
                            start=st, stop=sp,
                        )
                    nc.tensor.matmul(kps[:], wk_sb[:, f, :], ht_t[:],
                                     start=st, stop=sp)
                    nc.tensor.matmul(vps[:], wv_sb[:, f, :], ht_t[:],
                                     start=st, stop=sp)
                # Drain all six PSUM banks, alternating engines, so the next
                # tile's accumulation chains unblock as early as possible.
                qsb = []
                for i in range(HPC):
                    q = sb.tile([128, TT], BF16, tag="qsb", bufs=4,
                                name=f"qsb{i}")
                    if i % 2 == 0:
                        nc.scalar.copy(q[:], qps[i][:])
                    else:
                        nc.vector.tensor_copy(q[:], qps[i][:])
                    qsb.append(q)
                ksb = sb.tile([128, TT], BF16, tag="qsb", bufs=4, name="ksb")
                nc.scalar.copy(ksb[:], kps[:])
                nc.vector.tensor_copy(vT[b][:, ts(tt, TT)], vps[:])
                for h in range(HPC):
                    ln_rope(qsb[h], winvq_sb, cos_sb, sin_sb, tt, qT[(b, h)],
                            h % 2)
                ln_rope(ksb, winvk_sb, cos_sb, sin_sb, tt, kT[b], 0)

        def proj_vtrans(b):
            # transpose v to natural [j, d] chunk layout via the DMA xbar
            with nc.named_scope(f"proj_b{b}"):
                nc.scalar.dma_start_transpose(Vn[b][:], vT[b][:])

        def attn_heads(b, h0):
            """Two heads (h0, h0+1) interleaved; PV/ones lag scores by one
            pair so the PE stream never waits on the ACT exp."""
            with nc.named_scope(f"attn_b{b}"):
                streams = []
                for si, h in enumerate((h0, h0 + 1)):
                    attn_sb = sb.tile([128, S], BF16, tag=f"attn{si}", bufs=2,
                                      name=f"attn_sb{si}")
                    streams.append({"h": h, "si": si, "attn": attn_sb})

                def scores(st, qt, pp, kstart):
                    si, h = st["si"], st["h"]
                    i0 = qt * QT
                    kk = kstart + 2 * pp
                    j0 = i0 - WINDOW + kk * 128
                    sps = ps.tile([128, 2 * QT], F32,
                                  tag=f"acc{2 * si + pp % 2}", bufs=1,
                                  name="sps")
                    nc.tensor.matmul(
                        sps[:, 0:QT], kT[b][:, ds(j0, 128)],
                        qT[(b, h)][:, ds(i0, QT)],
                        start=True, stop=True,
                    )
                    nc.tensor.matmul(
                        sps[:, QT:2 * QT], kT[b][:, ds(j0 + 128, 128)],
                        qT[(b, h)][:, ds(i0, QT)],
                        start=True, stop=True,
                    )
                    pt = sb.tile([128, 2 * QT], BF16, tag=f"pt{si}", bufs=2,
                                 name="pt")
                    nc.scalar.activation(
                        pt[:], sps[:], mybir.ActivationFunctionType.Exp,
                        scale=SCALE,
                    )
                    if kk == 0:      # window edge pair (kk=0,1)
                        nc.vector.tensor_mul(pt[:], pt[:], maskw_sb[:])
                    elif kk == 4:    # causal edge pair (kk=4,5)
                        nc.vector.tensor_mul(pt[:], pt[:], maskc_sb[:])
                    return pt

                def pv(st, qt, pp, kstart, npair, pt, ops, lps):
                    i0 = qt * QT
                    kk = kstart + 2 * pp
                    j0 = i0 - WINDOW + kk * 128
                    first = pp == 0
                    last = pp == npair - 1
                    jc = j0 // 128
                    nc.tensor.matmul(
                        ops[:], Vn[b][:, jc, :], pt[:, 0:QT],
                        start=first, stop=False,
                    )
                    nc.tensor.matmul(
                        ops[:], Vn[b][:, jc + 1, :], pt[:, QT:2 * QT],
                        start=False, stop=last,
                    )
                    # softmax denominator, broadcast to all 128 partitions by
                    # the 128-identical-column ones stationary
                    nc.tensor.matmul(lps[:], ones_sb[:], pt[:, 0:QT],
                                     start=first, stop=False)
                    nc.tensor.matmul(lps[:], ones_sb[:], pt[:, QT:2 * QT],
                                     start=False, stop=last)

                def finish_qt(st, qt, ops, lps):
                    # Drain ops to SBUF immediately (frees the PSUM bank so
                    # the next tile's PV chain starts without waiting for the
                    # denominator), then normalize from SBUF.
                    si = st["si"]
                    i0 = qt * QT
                    osb = sb.tile([128, QT], F32, tag=f"osb{si}", bufs=2,
                                  name="osb")
                    nc.vector.tensor_copy(osb[:], ops[:])
                    linv = sb.tile([128, QT], F32, tag=f"linv{si}", bufs=2,
                                   name="linv")
                    # ~18-bit fast reciprocal keeps ACT free for the exps
                    # (full-precision DVE reciprocal is ~16 cyc/elem)
                    nc.vector.reciprocal_approx_fast(linv[:], lps[:])
                    nc.vector.tensor_mul(st["attn"][:, ds(i0, QT)], osb[:],
                                         linv[:])

                for qt in range(S // QT):
                    i0 = qt * QT
                    kstart = max(0, (WINDOW - i0) // 128)
                    npair = (NKC - kstart) // 2
                    ops = {}
                    lps = {}
                    for st in streams:
                        si = st["si"]
                        ops[si] = ps.tile([128, QT], F32,
                                          tag="acck" if si == 0 else "accv",
                                          bufs=1, name="ops")
                        lps[si] = ps.tile([128, QT], F32,
                                          tag="s0" if si == 0 else "s1",
                                          bufs=1, name="lps")
                    pts = {0: {}, 1: {}}
                    for pp in range(npair):
                        for st in streams:
                            pts[st["si"]][pp] = scores(st, qt, pp, kstart)
                        if pp > 0:
                            for st in streams:
                                si = st["si"]
                                pv(st, qt, pp - 1, kstart, npair,
                                   pts[si][pp - 1], ops[si], lps[si])
                                del pts[si][pp - 1]
                    for st in streams:
                        si = st["si"]
                        pv(st, qt, npair - 1, kstart, npair,
                           pts[si][npair - 1], ops[si], lps[si])
                    for st in streams:
                        finish_qt(st, qt, ops[st["si"]], lps[st["si"]])
                # ship both heads' outputs: one contiguous 64KB DMA per
                # (head, peer) on the gpsimd queue -- the collective that
                # consumes them lives there too, so ordering is natural and
                # the sync/scalar queues stay free for the next phase's
                # prefetches.
                for st in streams:
                    for n in range(NC):
                        nc.gpsimd.dma_start(
                            a2ain[b][n, st["h"]],
                            st["attn"][:, ts(n, TSL)],
                        )

        def a2a_phase(b):
            nc.gpsimd.collective_compute(
                "AllToAll",
                mybir.AluOpType.bypass,
                replica_groups=rg,
                ins=[a2ain[b][:]],
                outs=[a2aout[b][:]],
            )

        af = {}

        def af_load(b, eng):
            af[b] = sb.tile([128, NC, HPC, TSL], BF16, tag=f"af{b}", bufs=1,
                            name=f"af{b}")
            eng.dma_start(
                af[b][:], a2aout[b].rearrange("n c p t -> p n c t"),
            )

        PTAGS = ["acc0", "acc1", "acc2", "acc3", "acck", "accv", "s0", "s1"]

        def oproj_wo_load(obp):
            oblo, obhi = obp, obp + NOB // 2
            wo_lo = []
            wo_hi = []
            for i in range(FCH // 2):
                wl = sb.tile([128, 2, 512], BF16, tag="wolo", bufs=3,
                             name="wo_lo")
                wh = sb.tile([128, 2, 512], BF16, tag="wohi", bufs=3,
                             name="wo_hi")
                if i % 2 == 0:
                    nc.scalar.dma_start(wl[:], wo[oblo][:, ds(2 * i, 2), :])
                    nc.sync.dma_start(wh[:], wo[obhi][:, ds(2 * i, 2), :])
                else:
                    nc.sync.dma_start(wl[:], wo[oblo][:, ds(2 * i, 2), :])
                    nc.scalar.dma_start(wh[:], wo[obhi][:, ds(2 * i, 2), :])
                wo_lo.append(wl)
                wo_hi.append(wh)
            return wo_lo, wo_hi

        def oproj_obp(obp, groups, wo_pre=None):
            # one output-block pair (ob, ob+4) of o_proj for the given
            # (batch, t2) groups.  wo streams in contiguous 2-chunk pieces
            # alternating scalar/sync; each af stationary chunk feeds two
            # 512-wide matmuls (halved LDWEIGHTS tax).  c-outer piece order
            # means pieces are consumed sequentially -> small bufs stream.
            oblo, obhi = obp, obp + NOB // 2
            wo_lo, wo_hi = wo_pre if wo_pre else oproj_wo_load(obp)
            po = {}
            for gi, (bb, t2) in enumerate(groups):
                po[(bb, t2)] = (
                    ps.tile([128, 512], F32, tag=PTAGS[2 * gi], bufs=1,
                            name="plo"),
                    ps.tile([128, 512], F32, tag=PTAGS[2 * gi + 1], bufs=1,
                            name="phi"),
                )
            for pc in range(FCH // 2):
                for cc in range(2):
                    c = 2 * pc + cc
                    st = c == 0
                    sp = c == FCH - 1
                    for bb, t2 in groups:
                        stat = af[bb][:, c // HPC, c % HPC, ds(t2 * 128, 128)]
                        p_lo, p_hi = po[(bb, t2)]
                        nc.tensor.matmul(p_lo[:], stat, wo_lo[pc][:, cc, :],
                                         start=st, stop=sp)
                        nc.tensor.matmul(p_hi[:], stat, wo_hi[pc][:, cc, :],
                                         start=st, stop=sp)
            for gi, (bb, t2) in enumerate(groups):
                p_lo, p_hi = po[(bb, t2)]
                for ob, pp in ((oblo, p_lo), (obhi, p_hi)):
                    ot = sb.tile([128, 512], BF16, tag="ot", bufs=2,
                                 name="ot")
                    if (t2 + ob) % 2 == 0:
                        nc.scalar.copy(ot[:], pp[:])
                    else:
                        nc.vector.tensor_copy(ot[:], pp[:])
                    if ob % 2 == 0:
                        nc.sync.dma_start(out[bb, t2, ob], ot[:])
                    else:
                        nc.scalar.dma_start(out[bb, t2, ob], ot[:])

        def oproj_phase(wo_x):
            with nc.named_scope("oproj"):
                # batch-0-only pre-pass on obp0 (wo prefetched during attn1)
                # hides the batch-1 AllToAll; then the fused remainder
                # streams wo once for both batches
                oproj_obp(0, [(0, 0), (0, 1)], wo_pre=wo_x)
                oproj_obp(0, [(1, 0), (1, 1)])
                for obp in range(1, NOB // 2):
                    oproj_obp(obp, [(0, 0), (0, 1), (1, 0), (1, 1)])

        proj_setup(0)
        for tt in range(NTT):
            proj_tt(0, tt)
        proj_vtrans(0)
        proj_setup(1)    # cos/sin for batch 1 load during attn0
        for h0 in range(0, HPC, 2):
            attn_heads(0, h0)
        a2a_phase(0)
        for tt in range(NTT):
            proj_tt(1, tt)
        proj_vtrans(1)
        af_load(0, nc.sync)      # a2a0 done long ago
        wo_x = oproj_wo_load(0)  # X-pass wo streams during attn1 (idle DMA)
        for h0 in range(0, HPC, 2):
            attn_heads(1, h0)
        a2a_phase(1)
        af_load(1, nc.gpsimd)    # queued right behind a2a1 -> no HOL block
        oproj_phase(wo_x)

    nc.compile()
    return nc


def _prep_inputs(inputs):
    hidden = np.asarray(inputs["hidden_states"], np.float32)
    pos = np.asarray(inputs["position_ids"])
    cos = np.asarray(inputs["cos"], np.float32)
    sin = np.asarray(inputs["sin"], np.float32)
    wq = np.asarray(inputs["wq"], np.float32)
    wk = np.asarray(inputs["wk"], np.float32)
    wv = np.asarray(inputs["wv"], np.float32)
    wo = np.asarray(inputs["wo"], np.float32)
    qw = np.asarray(inputs["q_norm_w"], np.float32)
    kw = np.asarray(inputs["k_norm_w"], np.float32)

    # hT tiles: [B, FCH, NTT, 128, TT] contiguous per (b, f, tt)
    hTf = hidden.transpose(0, 2, 1).reshape(B, FCH, 128, NTT, TT)
    hT = np.ascontiguousarray(hTf.transpose(0, 1, 3, 2, 4)).astype(npbf16)
    cosT = np.ascontiguousarray(cos[pos].transpose(0, 2, 1)).astype(npbf16)
    sinT_f = sin[pos].transpose(0, 2, 1).copy()
    sinT_f[:, 0::2, :] *= -1.0
    sinT = np.ascontiguousarray(sinT_f).astype(npbf16)

    winvq = np.repeat(
        (1.0 / np.where(qw == 0, 1, qw) ** 2).reshape(D, 1), 128, axis=1
    ).astype(npbf16)
    winvk = np.repeat(
        (1.0 / np.where(kw == 0, 1, kw) ** 2).reshape(D, 1), 128, axis=1
    ).astype(npbf16)
    # wo: [NOB, 128, FCH, 512] (partition-major contiguous per ob block)
    woT = np.ascontiguousarray(
        wo.reshape(FCH, 128, NOB, 512).transpose(2, 1, 0, 3)
    ).astype(npbf16)

    def pmajor(w):
        # [HID, n] -> [128, FCH, n]
        n = w.shape[1]
        return np.ascontiguousarray(
            w.reshape(FCH, 128, n).transpose(1, 0, 2)
        ).astype(npbf16)

    in_maps = []
    for c in range(NC):
        wq_c = wq[:, c * QW:(c + 1) * QW].copy()
        for j in range(HPC):
            blk = wq_c[:, j * D:(j + 1) * D]
            blk -= blk.mean(axis=1, keepdims=True)
            blk *= qw[None, :]
        wk_c = wk[:, c * D:(c + 1) * D].copy()
        wk_c -= wk_c.mean(axis=1, keepdims=True)
        wk_c *= kw[None, :]
        in_maps.append({
            "hT": hT,
            "cosT": cosT,
            "sinT": sinT,
            "wq": pmajor(wq_c),
            "wk": pmajor(wk_c),
            "wv": pmajor(wv[:, c * D:(c + 1) * D]),
            "wo": woT,
            "winvq": winvq,
            "winvk": winvk,
        })
    return in_maps


def _run(inputs, **kwargs):
    if "nc" not in _CACHE:
        _CACHE["nc"] = _build_module()
    nc = _CACHE["nc"]
    in_maps = _prep_inputs(inputs)
    res = run_bass_kernel_spmd(nc, in_maps, core_ids=list(range(NC)), **kwargs)
    # core c holds out[b, :, :, :, :] for tokens c*TSL:(c+1)*TSL of each batch
    shards = []
    for c in range(NC):
        o = res.results[c]["out"].reshape(B, TSL // 128, NOB, 128, 512)
        # -> [B, TSL, HID]
        o = o.transpose(0, 1, 3, 2, 4).reshape(B, TSL, HID)
        shards.append(o)
    full = np.concatenate(shards, axis=1).astype(np.float32)
    return full, res


def kernel(**inputs) -> np.ndarray:
    out, _ = _run(inputs)
    return out


if __name__ == "__main__":
    import reference
    ins = {k: np.asarray(v) for k, v in reference.setup_inputs().items()}
    expected = np.asarray(reference.reference(**reference.setup_inputs()))
    actual = kernel(**ins)
    err = np.linalg.norm(actual - expected) / np.linalg.norm(expected)
    print("Relative error:", err)
